# revision 5
# baseline (speedup 1.0000x reference)
"""Sparse MoE kernel v4: token-parallel top-2 compute, software-pipelined.

Per core: 512 tokens (balanced assignment via host-side permutation so every
(core, expert) cell holds <= CAP tokens). Router logits via 3 bf16 hi/lo
matmul products (exact to ~3e-6, margin 1.1e-4). Per-expert compacted compute:
  rank[t] = strict-lower-tri @ flag       (exclusive cumsum, exact)
  P[t,c]  = flag * (rank == c)            (one-hot, bf16)
  gather/mm1/SwiGLU/mm2(d-major)/transpose/scatter, all matmuls.
The expert loop is software-pipelined: expert e+1's gather and one-hot
transposes are emitted inside expert e's dependency-wait windows so the PE
never idles (keeps the 2.4GHz p-state).
"""

import numpy as np
import ml_dtypes

P = 128
D_MODEL = 1024
D_FFN = 2048
N_EXPERTS = 8
B, S = 2, 2048
T_FULL = B * S
N_CORES = 8
T = T_FULL // N_CORES   # 512
DT = D_MODEL // P       # 8
FT = D_FFN // P         # 16
TT = T // P             # 4
FH = 2
DB = 2
LN_EPS = 1e-5

_CACHED = {}


def _build_nc(ln_trivial, CAP):
    import concourse.bacc as bacc
    import concourse.mybir as mybir
    import concourse.tile as tile
    import concourse.bass as bass

    CC = (CAP + P - 1) // P

    f32 = mybir.dt.float32
    bf16 = mybir.dt.bfloat16
    AF = mybir.ActivationFunctionType
    OP = mybir.AluOpType
    AX = mybir.AxisListType

    nc = bacc.Bacc()

    xth = nc.dram_tensor("xth", [D_MODEL, T], bf16, kind="ExternalInput")
    xtl = nc.dram_tensor("xtl", [D_MODEL, T], bf16, kind="ExternalInput")
    wrh = nc.dram_tensor("wrh", [P, DT * N_EXPERTS], bf16, kind="ExternalInput")
    wrl = nc.dram_tensor("wrl", [P, DT * N_EXPERTS], bf16, kind="ExternalInput")
    wgt = nc.dram_tensor("wgt", [N_EXPERTS, FH, 2, P, 4, D_FFN // FH], bf16, kind="ExternalInput")
    wut = nc.dram_tensor("wut", [N_EXPERTS, FH, 2, P, 4, D_FFN // FH], bf16, kind="ExternalInput")
    wdt = nc.dram_tensor("wdt", [N_EXPERTS, 4, P, 4, D_MODEL], bf16, kind="ExternalInput")
    gam = nc.dram_tensor("gam", [D_MODEL], f32, kind="ExternalInput")
    bet = nc.dram_tensor("bet", [D_MODEL], f32, kind="ExternalInput")
    out = nc.dram_tensor("out", [T, D_MODEL], bf16, kind="ExternalOutput")

    xth_r = xth.rearrange("(dt p) t -> dt p t", p=P)
    xtl_r = xtl.rearrange("(dt p) t -> dt p t", p=P)
    out_r = out.rearrange("(tt p) d -> tt p d", p=P)

    with tile.TileContext(nc) as tc:
        with (
            tc.tile_pool(name="consts", bufs=1) as consts,
            tc.tile_pool(name="xpool", bufs=1) as xpool,
            tc.tile_pool(name="rtr", bufs=2) as rtr,
            tc.tile_pool(name="wg", bufs=5) as wgp,
            tc.tile_pool(name="wu", bufs=5) as wup,
            tc.tile_pool(name="wd", bufs=5) as wdp,
            tc.tile_pool(name="hp", bufs=18) as hp,
            tc.tile_pool(name="sg", bufs=3) as sgp,
            tc.tile_pool(name="perm", bufs=2) as perm,
            tc.tile_pool(name="xep", bufs=2) as xep,
            tc.tile_pool(name="yep", bufs=1) as yep,
            tc.tile_pool(name="accp", bufs=1) as accp,
            tc.tile_pool(name="outp", bufs=2) as outp,
            tc.tile_pool(name="ps", bufs=8, space="PSUM") as ps,
        ):
            # ---- router inputs first in the DMA queue; wr comes packed as
            # one [P, DT*E] tile (single DMA with 128B lines)
            wrh_sb = consts.tile([P, DT * N_EXPERTS], bf16)
            nc.sync.dma_start(out=wrh_sb, in_=wrh.ap())
            wrl_sb = consts.tile([P, DT * N_EXPERTS], bf16)
            nc.sync.dma_start(out=wrl_sb, in_=wrl.ap())
            wrh_t = [wrh_sb[:, dt * N_EXPERTS : (dt + 1) * N_EXPERTS] for dt in range(DT)]
            wrl_t = [wrl_sb[:, dt * N_EXPERTS : (dt + 1) * N_EXPERTS] for dt in range(DT)]
            xh_t, xl_t = [], []
            for dt in range(DT):
                xh = xpool.tile([P, T], bf16, tag="xh", bufs=DT)
                nc.sync.dma_start(out=xh, in_=xth_r[dt])
                xh_t.append(xh)
                xl = xpool.tile([P, T], bf16, tag="xl", bufs=DT)
                nc.sync.dma_start(out=xl, in_=xtl_r[dt])
                xl_t.append(xl)

            # -- weight stream helpers (DMA queue order == emission order)
            wg_tiles = {}
            wu_tiles = {}
            wd_tiles = {}

            def load_gu(e, fh):
                for dth in range(2):
                    g = wgp.tile([P, 4, D_FFN // FH], bf16, tag="wg")
                    nc.sync.dma_start(out=g, in_=wgt[e, fh, dth])
                    wg_tiles[(e, fh, dth)] = g
                    u = wup.tile([P, 4, D_FFN // FH], bf16, tag="wu")
                    nc.sync.dma_start(out=u, in_=wut[e, fh, dth])
                    wu_tiles[(e, fh, dth)] = u

            def load_wd(e):
                for ftg in range(4):
                    w = wdp.tile([P, 4, D_MODEL], bf16, tag="wd")
                    nc.sync.dma_start(out=w, in_=wdt[e, ftg])
                    wd_tiles[(e, ftg)] = w

            # -- constants built on the idle GPSIMD engine (no DMA traffic)
            idn_sb = consts.tile([P, P], bf16)
            nc.gpsimd.memset(idn_sb, 0.0)
            nc.gpsimd.affine_select(
                out=idn_sb, in_=idn_sb, compare_op=OP.not_equal, fill=1.0,
                base=0, pattern=[[-1, P]], channel_multiplier=1,
            )
            idf_sb = consts.tile([N_EXPERTS, N_EXPERTS], f32)
            nc.gpsimd.memset(idf_sb, 0.0)
            nc.gpsimd.affine_select(
                out=idf_sb, in_=idf_sb, compare_op=OP.not_equal, fill=1.0,
                base=0, pattern=[[-1, N_EXPERTS]], channel_multiplier=1,
            )
            tri_sb = consts.tile([P, TT, T], bf16)
            for kt in range(TT):
                nc.gpsimd.memset(tri_sb[:, kt, :], 0.0)
                nc.gpsimd.affine_select(
                    out=tri_sb[:, kt, :], in_=tri_sb[:, kt, :],
                    compare_op=OP.is_ge, fill=1.0,
                    base=kt * P, pattern=[[-1, T]], channel_multiplier=1,
                )
            ioc_sb = consts.tile([P, CAP], f32)
            nc.gpsimd.iota(
                ioc_sb, pattern=[[1, CAP]], base=0, channel_multiplier=0,
                allow_small_or_imprecise_dtypes=True,
            )
            load_gu(0, 0)
            if not ln_trivial:
                gam_sb = consts.tile([P, D_MODEL], f32)
                bet_sb = consts.tile([P, D_MODEL], f32)
                nc.sync.dma_start(
                    out=gam_sb, in_=bass.AP(tensor=gam.ap().tensor, offset=0, ap=[[0, P], [1, D_MODEL]])
                )
                nc.sync.dma_start(
                    out=bet_sb, in_=bass.AP(tensor=bet.ap().tensor, offset=0, ap=[[0, P], [1, D_MODEL]])
                )
            eps_sb = consts.tile([P, 1], f32)
            nc.vector.memset(eps_sb, LN_EPS)
            scale_sb = consts.tile([P, TT, N_EXPERTS], f32)

            # rest of expert-0 weights
            load_gu(0, 1)
            load_wd(0)

            # ---- router: logits in [e, t] form via 3 bf16 hi/lo products
            pr = ps.tile([N_EXPERTS, T], f32, tag="pb", bufs=2)
            prods = [(wrh_t, xh_t), (wrh_t, xl_t), (wrl_t, xh_t)]
            n_mm = len(prods) * DT
            i = 0
            for wt, xt in prods:
                for dt in range(DT):
                    nc.tensor.matmul(
                        pr, lhsT=wt[dt], rhs=xt[dt],
                        start=(i == 0), stop=(i == n_mm - 1),
                    )
                    i += 1
            lgT = rtr.tile([N_EXPERTS, T], f32, tag="lgT", bufs=1)
            nc.vector.tensor_copy(lgT, pr)
            lgs = consts.tile([P, TT, N_EXPERTS], f32)
            mstore = consts.tile([P, TT, 2], f32)
            # derive xn (token-major bf16 x) from xh by PE transpose; runs in
            # the DMA-paced startup window
            xn_sb = xpool.tile([P, TT, D_MODEL], bf16)
            for kt in range(TT):
                for dt in range(DT):
                    ptx = ps.tile([P, P], bf16, tag="pa", bufs=2)
                    nc.tensor.transpose(
                        ptx, xh_t[dt][:, kt * P : (kt + 1) * P], idn_sb
                    )
                    nc.scalar.activation(
                        xn_sb[:, kt, dt * P : (dt + 1) * P], ptx, AF.Copy
                    )
            # pass 1: flags (membership) only — shortest path to rank/gather
            flagb = perm.tile([P, TT, N_EXPERTS], bf16, tag="flagb", bufs=1)
            flagf = perm.tile([P, TT, N_EXPERTS], f32, tag="flagf", bufs=1)
            for tt in range(TT):
                ptl = ps.tile([P, N_EXPERTS], f32, tag="pa", bufs=2)
                nc.tensor.transpose(
                    ptl, lgT[:, tt * P : (tt + 1) * P], idf_sb
                )
                lg = lgs[:, tt, :]
                nc.vector.tensor_copy(lg, ptl)
                m1 = mstore[:, tt, 0:1]
                nc.vector.reduce_max(m1, lg, axis=AX.X)
                msk = rtr.tile([P, N_EXPERTS], f32, tag="msk")
                nc.vector.tensor_scalar(msk, lg, scalar1=m1, scalar2=None, op0=OP.is_equal)
                nc.vector.tensor_scalar(msk, msk, scalar1=-1e30, scalar2=None, op0=OP.mult)
                nc.vector.tensor_add(msk, msk, lg)
                m2 = mstore[:, tt, 1:2]
                nc.vector.reduce_max(m2, msk, axis=AX.X)
                nc.vector.tensor_scalar(
                    flagf[:, tt, :], lg, scalar1=m2, scalar2=None, op0=OP.is_ge,
                )
                nc.vector.tensor_copy(flagb[:, tt, :], flagf[:, tt, :])
            rank = perm.tile([P, TT, N_EXPERTS], f32, tag="rank", bufs=1)
            for mt in range(TT):
                prk = ps.tile([P, N_EXPERTS], f32, tag="pa", bufs=2)
                for kt in range(TT):
                    nc.tensor.matmul(
                        prk, lhsT=tri_sb[:, kt, mt * P : (mt + 1) * P],
                        rhs=flagb[:, kt, :],
                        start=(kt == 0), stop=(kt == TT - 1),
                    )
                nc.vector.tensor_copy(rank[:, mt, :], prk)
            # pass 2: softmax weights (trails; needed only by pes builds)
            for tt in range(TT):
                lg = lgs[:, tt, :]
                m1 = mstore[:, tt, 0:1]
                m2 = mstore[:, tt, 1:2]
                eq1 = rtr.tile([P, N_EXPERTS], f32, tag="eq1")
                nc.vector.tensor_scalar(eq1, lg, scalar1=m1, scalar2=None, op0=OP.is_equal)
                eq2 = rtr.tile([P, N_EXPERTS], f32, tag="eq2")
                nc.vector.tensor_scalar(eq2, lg, scalar1=m2, scalar2=None, op0=OP.is_equal)
                d21 = rtr.tile([P, 1], f32, tag="d21")
                nc.vector.tensor_sub(d21, m2, m1)
                ex = rtr.tile([P, 1], f32, tag="ex")
                nc.scalar.activation(ex, d21, AF.Exp)
                den = rtr.tile([P, 1], f32, tag="den")
                nc.vector.tensor_scalar(den, ex, scalar1=1.0, scalar2=None, op0=OP.add)
                w1 = rtr.tile([P, 1], f32, tag="w1")
                nc.vector.reciprocal(w1, den)
                w2 = rtr.tile([P, 1], f32, tag="w2")
                nc.vector.tensor_mul(w2, ex, w1)
                nc.vector.tensor_scalar_mul(eq1, eq1, w1)
                nc.vector.tensor_scalar_mul(eq2, eq2, w2)
                nc.vector.tensor_add(scale_sb[:, tt, :], eq1, eq2)

            # ---- per-expert pipeline stages
            def build_onehot(e):
                pe = perm.tile([P, TT, CAP], bf16, tag="pe")
                pes = perm.tile([P, TT, CAP], bf16, tag="pes")
                for tt in range(TT):
                    eqc = rtr.tile([P, CAP], f32, tag="eqc")
                    nc.vector.tensor_scalar(
                        eqc, ioc_sb, scalar1=rank[:, tt, e : e + 1], scalar2=None,
                        op0=OP.is_equal,
                    )
                    nc.vector.tensor_scalar(
                        pe[:, tt, :], eqc, scalar1=flagf[:, tt, e : e + 1], scalar2=None,
                        op0=OP.mult,
                    )
                    nc.vector.tensor_scalar(
                        pes[:, tt, :], eqc, scalar1=scale_sb[:, tt, e : e + 1],
                        scalar2=None, op0=OP.mult,
                    )
                return pe, pes

            def trans_onehot(pes):
                pet = perm.tile([P, CC, T], bf16, tag="pet")
                for cc in range(CC):
                    cw = min(P, CAP - cc * P)
                    for tt in range(TT):
                        ptp = ps.tile([P, P], bf16, tag="pa", bufs=2)
                        nc.tensor.transpose(
                            ptp[:cw, :], pes[:, tt, cc * P : cc * P + cw], idn_sb
                        )
                        nc.scalar.activation(
                            pet[:cw, cc, tt * P : (tt + 1) * P], ptp[:cw, :], AF.Copy
                        )
                return pet

            def gather_half(e, pe, xg, half):
                for dt in range(half * (DT // 2), (half + 1) * (DT // 2)):
                    pxg = ps.tile([P, CAP], f32, tag="pa", bufs=2)
                    for kt in range(TT):
                        nc.tensor.matmul(
                            pxg, lhsT=xn_sb[:, kt, dt * P : (dt + 1) * P],
                            rhs=pe[:, kt, :],
                            start=(kt == 0), stop=(kt == TT - 1),
                        )
                    nc.scalar.activation(xg[:, dt, :], pxg, AF.Copy)

            def ln_tt(tt):
                a = acc[:, tt, :]
                a2 = a.rearrange("p (s f) -> p s f", s=2)
                stats = rtr.tile([P, 2, 6], f32, tag="stats")
                for s_ in range(2):
                    nc.vector.bn_stats(out=stats[:, s_, :], in_=a2[:, s_, :])
                mv = rtr.tile([P, 2], f32, tag="mv")
                nc.vector.bn_aggr(out=mv, in_=stats)
                mean = mv[:, 0:1]
                rstd = rtr.tile([P, 1], f32, tag="rstd")
                nc.scalar.activation(
                    rstd, mv[:, 1:2], AF.Sqrt, bias=eps_sb, scale=1.0, alpha=0.0
                )
                nc.vector.reciprocal(rstd, rstd)
                o_sb = outp.tile([P, D_MODEL], bf16, tag="o")
                if ln_trivial:
                    nc.vector.tensor_scalar(
                        o_sb, a, scalar1=mean, scalar2=rstd,
                        op0=OP.subtract, op1=OP.mult,
                    )
                else:
                    of = outp.tile([P, D_MODEL], f32, tag="of")
                    nc.vector.tensor_scalar(
                        of, a, scalar1=mean, scalar2=rstd,
                        op0=OP.subtract, op1=OP.mult,
                    )
                    nc.vector.tensor_mul(of, of, gam_sb)
                    nc.vector.tensor_add(o_sb, of, bet_sb)
                nc.sync.dma_start(out=out_r[tt], in_=o_sb)

            # state carried between pipeline iterations
            pe_c, pes_c = build_onehot(0)
            pet_c = trans_onehot(pes_c)
            xg_c = xep.tile([P, DT, CAP], bf16, tag="xg")
            gather_half(0, pe_c, xg_c, 0)
            gather_half(0, pe_c, xg_c, 1)

            acc = accp.tile([P, TT, D_MODEL], f32)
            for e in range(N_EXPERTS):
                # queue next expert's weights behind this one's
                if e + 1 < N_EXPERTS:
                    if e + 1 == N_EXPERTS - 1:
                        load_gu(e + 1, 0)
                        load_wd(e + 1)
                        load_gu(e + 1, 1)
                    else:
                        load_gu(e + 1, 0)
                        load_gu(e + 1, 1)
                        load_wd(e + 1)
                    pe_n, pes_n = build_onehot(e + 1)   # DVE, overlaps mm1
                    xg_n = xep.tile([P, DT, CAP], bf16, tag="xg")
                else:
                    pe_n = pes_n = xg_n = None

                # -- mm1 + SwiGLU on capacity tokens
                last = e == N_EXPERTS - 1
                hs = []

                def mm1_fts(ft0, ft1):
                    for ft in range(ft0, ft1):
                        fh, fi = divmod(ft, FT // FH)
                        pg = ps.tile([P, CAP], f32, tag="pg", bufs=2)
                        pu = ps.tile([P, CAP], f32, tag="pu", bufs=2)
                        for dt in range(DT):
                            nc.tensor.matmul(
                                pg,
                                lhsT=wg_tiles[(e, fh, dt // 4)][:, dt % 4, fi * P : (fi + 1) * P],
                                rhs=xg_c[:, dt, :],
                                start=(dt == 0), stop=(dt == DT - 1),
                            )
                        for dt in range(DT):
                            nc.tensor.matmul(
                                pu,
                                lhsT=wu_tiles[(e, fh, dt // 4)][:, dt % 4, fi * P : (fi + 1) * P],
                                rhs=xg_c[:, dt, :],
                                start=(dt == 0), stop=(dt == DT - 1),
                            )
                        sg = sgp.tile([P, CAP], f32, tag="sg")
                        nc.scalar.activation(sg, pg, AF.Silu)
                        h = hp.tile([P, CAP], bf16, tag="h")
                        nc.vector.tensor_mul(h, sg, pu)
                        hs.append(h)

                yet = yep.tile([P, DT, CAP], bf16, tag="yet")
                if not last:
                    mm1_fts(0, FT)
                    # -- fill the h-tail window: first half of next gather
                    if pe_n is not None:
                        gather_half(e + 1, pe_n, xg_n, 0)
                    # -- mm2 in d-major form: yeT[d, c]
                    for dt in range(DT):
                        pyt = ps.tile([P, CAP], f32, tag="pb", bufs=2)
                        for ft in range(FT):
                            nc.tensor.matmul(
                                pyt,
                                lhsT=wd_tiles[(e, ft // 4)][:, ft % 4, dt * P : (dt + 1) * P],
                                rhs=hs[ft],
                                start=(ft == 0), stop=(ft == FT - 1),
                            )
                        nc.scalar.activation(yet[:, dt, :], pyt, AF.Copy)
                    # -- second half of next gather fills the yet window
                    if pe_n is not None:
                        gather_half(e + 1, pe_n, xg_n, 1)
                else:
                    # last expert: mm2 first half overlaps the tail of the
                    # weight stream (wd arrives between the two gate/up halves)
                    mm1_fts(0, FT // 2)
                    yh1 = yep.tile([P, DT, CAP], bf16, tag="yh1")
                    for dt in range(DT):
                        pyt = ps.tile([P, CAP], f32, tag="pb", bufs=2)
                        for ft in range(FT // 2):
                            nc.tensor.matmul(
                                pyt,
                                lhsT=wd_tiles[(e, ft // 4)][:, ft % 4, dt * P : (dt + 1) * P],
                                rhs=hs[ft],
                                start=(ft == 0), stop=(ft == FT // 2 - 1),
                            )
                        nc.scalar.activation(yh1[:, dt, :], pyt, AF.Copy)
                    mm1_fts(FT // 2, FT)
                    for dt in range(DT):
                        pyt = ps.tile([P, CAP], f32, tag="pb", bufs=2)
                        for ft in range(FT // 2, FT):
                            nc.tensor.matmul(
                                pyt,
                                lhsT=wd_tiles[(e, ft // 4)][:, ft % 4, dt * P : (dt + 1) * P],
                                rhs=hs[ft],
                                start=(ft == FT // 2), stop=(ft == FT - 1),
                            )
                        nc.vector.tensor_add(yet[:, dt, :], yh1[:, dt, :], pyt)

                # -- transpose yeT -> ye[c, d]; dt-halves first so the scatter
                # can start on db=0 after 8 transposes
                ye = yep.tile([P, CC, D_MODEL], bf16, tag="ye")
                for dhalf in range(2):
                    for cc in range(CC):
                        cw = min(P, CAP - cc * P)
                        for dt in range(dhalf * (DT // 2), (dhalf + 1) * (DT // 2)):
                            ptp2 = ps.tile([P, P], bf16, tag="pa", bufs=2)
                            nc.tensor.transpose(
                                ptp2[:cw, :], yet[:, dt, cc * P : cc * P + cw], idn_sb
                            )
                            nc.scalar.activation(
                                ye[:cw, cc, dt * P : (dt + 1) * P], ptp2[:cw, :], AF.Copy
                            )

                # -- next expert's one-hot transposes fill the ye window
                if pe_n is not None:
                    pet_n = trans_onehot(pes_n)
                else:
                    pet_n = None

                # -- scatter: y[t, d] += sum_c PT[c, t] * ye[c, d]
                # psum rotates over three tags (6 banks) so the PE never waits
                # on the trailing DVE accumulate
                sc_tags = ["pb", "pg", "pu"]
                gi = 0
                order = [(tt, db) for tt in range(TT) for db in range(DB)] if last \
                    else [(tt, db) for db in range(DB) for tt in range(TT)]
                for tt, db in order:
                    psc = ps.tile([P, 512], f32, tag=sc_tags[gi % 3], bufs=2)
                    gi += 1
                    for cc in range(CC):
                        cw = min(P, CAP - cc * P)
                        nc.tensor.matmul(
                            psc, lhsT=pet_c[:cw, cc, tt * P : (tt + 1) * P],
                            rhs=ye[:cw, cc, db * 512 : (db + 1) * 512],
                            start=(cc == 0), stop=(cc == CC - 1),
                        )
                    dst = acc[:, tt, db * 512 : (db + 1) * 512]
                    if e == 0:
                        nc.vector.tensor_copy(dst, psc)
                    else:
                        nc.vector.tensor_add(dst, dst, psc)
                    if last and db == DB - 1:
                        ln_tt(tt)

                pe_c, pes_c, pet_c, xg_c = pe_n, pes_n, pet_n, xg_n

    nc.finalize()
    return nc


def _routing_top2(x, w_router):
    logits = x @ w_router.T
    part = np.argpartition(-logits, 2, axis=1)[:, :2]
    return part


def _balance_perm(top2):
    rng = np.random.default_rng(0)
    best = None
    for trial in range(8):
        order = rng.permutation(T_FULL)
        counts = np.zeros((N_CORES, N_EXPERTS), np.int32)
        fill = np.zeros(N_CORES, np.int32)
        assign = np.full(T_FULL, -1, np.int32)
        for t in order:
            e1, e2 = top2[t]
            bestc, bestkey = -1, None
            for c in range(N_CORES):
                if fill[c] >= T:
                    continue
                key = (max(counts[c, e1] + 1, counts[c, e2] + 1),
                       counts[c, e1] + counts[c, e2], fill[c])
                if bestkey is None or key < bestkey:
                    bestkey, bestc = key, c
            assign[t] = bestc
            counts[bestc, e1] += 1
            counts[bestc, e2] += 1
            fill[bestc] += 1
        m = int(counts.max())
        if best is None or m < best[0]:
            best = (m, assign.copy())
        if m <= 134:
            break
    m, assign = best
    perm = np.concatenate([np.where(assign == c)[0] for c in range(N_CORES)])
    return perm, m


def _choose_perm_cap(inputs):
    x2 = np.asarray(inputs["x"], dtype=np.float32).reshape(T_FULL, D_MODEL)
    top2 = _routing_top2(x2, np.asarray(inputs["w_router"], dtype=np.float32))
    perm, maxcell = _balance_perm(top2)
    if maxcell <= 136:
        return perm, 136
    perm = np.arange(T_FULL)
    counts = np.zeros((N_CORES, N_EXPERTS), np.int32)
    for c in range(N_CORES):
        for t in range(c * T, (c + 1) * T):
            counts[c, top2[t, 0]] += 1
            counts[c, top2[t, 1]] += 1
    mc = int(counts.max())
    return perm, max(160, ((mc + 31) // 32) * 32)


def build_in_maps(inputs, perm=None, cap=None):
    if perm is None or cap is None:
        perm, cap = _choose_perm_cap(inputs)
    x = np.asarray(inputs["x"], dtype=np.float32).reshape(T_FULL, D_MODEL)
    x = x[perm]
    w_router = np.asarray(inputs["w_router"], dtype=np.float32)
    w_gate = np.asarray(inputs["w_gate"], dtype=np.float32)
    w_up = np.asarray(inputs["w_up"], dtype=np.float32)
    w_down = np.asarray(inputs["w_down"], dtype=np.float32)
    ln_gamma = np.asarray(inputs["ln_gamma"], dtype=np.float32)
    ln_beta = np.asarray(inputs["ln_beta"], dtype=np.float32)

    bf = ml_dtypes.bfloat16
    def pack_gu(w):
        # [E, F, D] -> w.T per expert [D, F] -> [E, FH, DTH, P, 4, F/FH]
        wt = np.ascontiguousarray(w.transpose(0, 2, 1)).astype(bf)   # [E, D, F]
        wt = wt.reshape(N_EXPERTS, 2, 4, P, FH, D_FFN // FH)
        return np.ascontiguousarray(wt.transpose(0, 4, 1, 3, 2, 5))
    wgt = pack_gu(w_gate)
    wut = pack_gu(w_up)
    wdt = np.ascontiguousarray(w_down.transpose(0, 2, 1)).astype(bf)   # [E, F, D]
    wdt = np.ascontiguousarray(
        wdt.reshape(N_EXPERTS, 4, 4, P, D_MODEL).transpose(0, 1, 3, 2, 4)
    )
    wrt = np.ascontiguousarray(w_router.T)           # [D, E] f32
    wrh = wrt.astype(bf)
    wrl = (wrt - wrh.astype(np.float32)).astype(bf)
    # pack [D, E] -> [P, DT*E] so each partition line is contiguous
    wrh = np.ascontiguousarray(wrh.reshape(DT, P, N_EXPERTS).transpose(1, 0, 2).reshape(P, DT * N_EXPERTS))
    wrl = np.ascontiguousarray(wrl.reshape(DT, P, N_EXPERTS).transpose(1, 0, 2).reshape(P, DT * N_EXPERTS))

    in_maps = []
    for c in range(N_CORES):
        xs = x[c * T : (c + 1) * T]
        xst = np.ascontiguousarray(xs.T)             # [D, T] f32
        xth = xst.astype(bf)
        xtl = (xst - xth.astype(np.float32)).astype(bf)
        in_maps.append({
            "xth": xth,
            "xtl": xtl,
            "wrh": wrh,
            "wrl": wrl,
            "wgt": wgt,
            "wut": wut,
            "wdt": wdt,
            "gam": ln_gamma,
            "bet": ln_beta,
        })
    return in_maps


def kernel(**inputs) -> np.ndarray:
    from concourse.bass_utils import run_bass_kernel_spmd

    ln_trivial = bool(
        np.all(np.asarray(inputs["ln_gamma"]) == 1.0)
        and np.all(np.asarray(inputs["ln_beta"]) == 0.0)
    )
    perm, cap = _choose_perm_cap(inputs)
    in_maps = build_in_maps(inputs, perm, cap)
    key = ("nc", ln_trivial, cap)
    if key not in _CACHED:
        _CACHED[key] = _build_nc(ln_trivial, cap)
    _CACHED["nc"] = _CACHED[key]
    res = run_bass_kernel_spmd(_CACHED[key], in_maps, core_ids=list(range(N_CORES)))
    out = np.concatenate(
        [res.results[c]["out"].astype(np.float32) for c in range(N_CORES)], axis=0
    )
    inv = np.empty_like(perm)
    inv[perm] = np.arange(T_FULL)
    return out[inv].reshape(B, S, D_MODEL)
